# revision 8
# baseline (speedup 1.0000x reference)
"""Tensor-parallel LlamaAttention (B=1, S=2048, H=4096, 32 q-heads / 8 kv-heads,
head_dim=128) on 8 Trainium2 NeuronCores.

Sharding: core c owns query heads 4c..4c+3 and KV head c (GQA group), i.e.
Wq rows [512c, 512c+512), Wk/Wv rows [128c, 128c+128), and Wo columns
[512c, 512c+512). Each core produces a full-shape [2048, 4096] partial of the
output projection; the host sums the 8 partials.

All device-side matmuls run in "transposed" layouts so no large on-device
transposes are needed:
  - scores are computed as S.T[k, q] (k on partitions) so softmax needs no
    row-max (scores are O(1), exp cannot overflow) and the denominator is a
    ones-vector matmul over the partition dim.
  - attention output comes out as attnT[d, q], which is exactly the stationary
    operand layout the output projection needs.
"""

import math
import sys

sys.path.insert(0, "/opt/trn_rl_repo")

import numpy as np

import concourse.bass as bass
import concourse.mybir as mybir
import concourse.tile as tile_mod
from concourse.tile import ScopedClock

F32 = mybir.dt.float32

S = 2048
H = 4096
DQ = 512  # per-core query width (4 heads x 128)
DKV = 128  # per-core kv width (1 head)
D = 128  # head dim
N_CORES = 8
HEADS = 4  # q heads per core
ROPE_THETA = 500000.0
SM_SCALE = 1.0 / math.sqrt(D)

HT = H // 128  # 32 contraction tiles
ST_A = 256  # pass-A moving-operand width
N_ST_A = S // ST_A
QT_W = 512  # phase-B q-tile width
N_QT = S // QT_W
N_KT = S // 128  # 16 k-tiles of 128
ET = 512  # phase-C output e-tile width


def _patch_tilecontext():
    """walrus's CTRL codegen rejects >2 sync waits on one instruction; the
    Tile kernel-tail drain waits on the whole global clock. Spread the waits
    one-per-nop before the drain."""
    if getattr(tile_mod.TileContext, "_drain_patched", False):
        return

    def _drain_and_barrier(self, tick_clock, wait_clock):
        nc = self.nc
        probe = nc.sync.nop(nofuse=True)
        wait_clock.add_sem_waits(
            probe.ins, ScopedClock({None: tick_clock.global_clock})
        )
        si = probe.ins.sync_info
        waits = list(si.on_wait or [])
        if len(waits) > 1:
            si.on_wait = waits[:1]
            for w in waits[1:]:
                n = nc.sync.nop(nofuse=True)
                if n.ins.sync_info is None:
                    n.ins.sync_info = mybir.SyncInfo(on_wait=[w], on_update=[])
                else:
                    n.ins.sync_info.on_wait = [w]
        nc.sync.drain()
        nc.all_engine_barrier()
        assert self.sems is not None
        popped = nc._tile_sem_poison_stack.pop()
        assert popped is self._sem_poison
        nc.clear_and_free_semaphores(list(self.sems.allocated().values()))
        nc.all_engine_barrier()

    tile_mod.TileContext._drain_and_barrier = _drain_and_barrier
    tile_mod.TileContext._drain_patched = True


def _split_sync_waits(nc, cap=1):
    """walrus's CoreV3 codegen rejects instructions carrying more than ~2
    sync-wait commands. Hoist extra waits onto nops inserted just before the
    instruction on the same engine (sound: Tile data-dep waits are
    sem-ge-imm, i.e. monotone)."""
    n_split = 0
    for fn in nc.m.functions:
        for bb in fn.blocks:
            new_insts = []
            for inst in bb.instructions:
                si = inst.sync_info
                waits = list(si.on_wait) if si and si.on_wait else []
                if len(waits) > cap:
                    keep = waits[-cap:]
                    for j, w in enumerate(waits[:-cap]):
                        nop = mybir.InstNoOp(
                            name=f"{inst.name}-wsplit{j}", ins=[], outs=[]
                        )
                        nop.engine = inst.engine
                        nop.sync_info = mybir.SyncInfo(on_wait=[w], on_update=[])
                        new_insts.append(nop)
                        n_split += 1
                    si.on_wait = keep
                new_insts.append(inst)
            bb.instructions[:] = new_insts
    return n_split


def _rope_epilogue(nc, pool, ps, out_ap, cos_ap, sin_ap, width):
    """out = ps * cos + rotate_half(ps) * sin_signed, straight out of PSUM.

    sin_ap carries the sign fold: rows 0:64 hold -sin, rows 64:128 hold +sin,
    so rotate_half is just a 64-partition swap on the ps read."""
    t1 = pool.tile([128, width], F32, tag="rope_t1")
    t2 = pool.tile([128, width], F32, tag="rope_t2")
    nc.vector.tensor_mul(t1[:], ps[:], cos_ap)
    nc.vector.tensor_mul(t2[0:64, :], ps[64:128, :], sin_ap[0:64, :])
    nc.vector.tensor_mul(t2[64:128, :], ps[0:64, :], sin_ap[64:128, :])
    nc.vector.tensor_add(out_ap, t1[:], t2[:])


def _build_program():
    _patch_tilecontext()
    nc = bass.Bass()

    xT = nc.declare_dram_parameter("xT", [H, S], F32, isOutput=False)
    wqT = nc.declare_dram_parameter("wqT", [H, DQ], F32, isOutput=False)
    wkT = nc.declare_dram_parameter("wkT", [H, DKV], F32, isOutput=False)
    wvT = nc.declare_dram_parameter("wvT", [H, DKV], F32, isOutput=False)
    woT = nc.declare_dram_parameter("woT", [DQ, H], F32, isOutput=False)
    cosT = nc.declare_dram_parameter("cosT", [D, S], F32, isOutput=False)
    sinT = nc.declare_dram_parameter("sinT", [D, S], F32, isOutput=False)
    masks = nc.declare_dram_parameter("masks", [128, 4 * QT_W], F32, isOutput=False)
    ident = nc.declare_dram_parameter("ident", [128, 128], F32, isOutput=False)
    ones = nc.declare_dram_parameter("ones", [128, 128], F32, isOutput=False)
    out = nc.declare_dram_parameter("out", [S, H], F32, isOutput=True)

    qT_dram = nc.dram_tensor("qT_scratch", [HEADS, D, S], F32)

    xT_t = xT[:].rearrange("(ht p) s -> p ht s", p=128)
    wqT_t = wqT[:].rearrange("(ht p) d -> p ht d", p=128)
    wkT_t = wkT[:].rearrange("(ht p) d -> p ht d", p=128)
    wvT_t = wvT[:].rearrange("(ht p) d -> p ht d", p=128)
    woT_t = woT[:].rearrange("(j p) e -> p j e", p=128)

    from contextlib import ExitStack

    with tile_mod.TileContext(nc) as tc:
        with ExitStack() as _stk:
            persist = _stk.enter_context(tc.tile_pool(name="persist", bufs=1))
            kt_sb = persist.tile([128, S], F32)  # K.T, rope'd (d x k)
            v_sb = persist.tile([128, N_KT, 128], F32)  # V natural (k x d) tiles
            ones_sb = persist.tile([128, 128], F32)
            nc.sync.dma_start(out=ones_sb[:], in_=ones[:])

            with tc.tile_pool(name="cs", bufs=1) as cs:
                cos_sb = cs.tile([128, S], F32, tag="cos")
                sin_sb = cs.tile([128, S], F32, tag="sin")
                nc.sync.dma_start(out=cos_sb[:], in_=cosT[:])
                nc.sync.dma_start(out=sin_sb[:], in_=sinT[:])

                # ---- Pass A0: K.T/V.T projections (+rope on K), V transpose
                with tc.tile_pool(name="a0", bufs=1) as a0, \
                     tc.tile_pool(name="a0_xt", bufs=2) as a0_xt, \
                     tc.tile_pool(name="a0_tmp", bufs=3) as a0_tmp, \
                     tc.tile_pool(name="a0_ps", bufs=4, space="PSUM") as a0_ps, \
                     tc.tile_pool(name="a0_tps", bufs=2, space="PSUM") as a0_tps:
                    ident_sb = a0.tile([128, 128], F32, tag="ident")
                    nc.sync.dma_start(out=ident_sb[:], in_=ident[:])
                    wk_sb = a0.tile([128, HT, DKV], F32, tag="wk")
                    wv_sb = a0.tile([128, HT, DKV], F32, tag="wv")
                    nc.sync.dma_start(out=wk_sb[:], in_=wkT_t)
                    nc.sync.dma_start(out=wv_sb[:], in_=wvT_t)
                    vt_sb = a0.tile([128, S], F32, tag="vt")  # V.T staging

                    for st in range(N_ST_A):
                        ssl = bass.ts(st, ST_A)
                        xt = a0_xt.tile([128, HT, ST_A], F32, tag="xt")
                        nc.sync.dma_start(out=xt[:], in_=xT_t[:, :, ssl])
                        for which in ("k", "v"):
                            w_sb = wk_sb if which == "k" else wv_sb
                            ps = a0_ps.tile([128, ST_A], F32, tag="ps")
                            for ht in range(HT):
                                nc.tensor.matmul(
                                    ps[:], w_sb[:, ht, :], xt[:, ht, :],
                                    start=(ht == 0), stop=(ht == HT - 1),
                                )
                            if which == "k":
                                _rope_epilogue(
                                    nc, a0_tmp, ps, kt_sb[:, ssl],
                                    cos_sb[:, ssl], sin_sb[:, ssl], ST_A,
                                )
                            else:
                                nc.vector.tensor_copy(vt_sb[:, ssl], ps[:])

                    for ki in range(N_KT):
                        tp = a0_tps.tile([128, 128], F32, tag="tp")
                        nc.tensor.transpose(
                            tp[:], vt_sb[:, bass.ts(ki, 128)], ident_sb[:]
                        )
                        nc.vector.tensor_copy(v_sb[:, ki, :], tp[:])

                # ---- Pass A1: Q.T projection (+rope), staged to DRAM
                with tc.tile_pool(name="a1_wq", bufs=1) as a1_wq, \
                     tc.tile_pool(name="a1_xt", bufs=2) as a1_xt, \
                     tc.tile_pool(name="a1_st", bufs=3) as a1_st, \
                     tc.tile_pool(name="a1_ps", bufs=4, space="PSUM") as a1_ps:
                    wq_sb = a1_wq.tile([128, HT, DQ], F32, tag="wq")
                    nc.sync.dma_start(out=wq_sb[:], in_=wqT_t)
                    for st in range(N_ST_A):
                        ssl = bass.ts(st, ST_A)
                        xt = a1_xt.tile([128, HT, ST_A], F32, tag="xt")
                        nc.sync.dma_start(out=xt[:], in_=xT_t[:, :, ssl])
                        for h in range(HEADS):
                            ps = a1_ps.tile([128, ST_A], F32, tag="ps")
                            for ht in range(HT):
                                nc.tensor.matmul(
                                    ps[:],
                                    wq_sb[:, ht, bass.ts(h, D)],
                                    xt[:, ht, :],
                                    start=(ht == 0), stop=(ht == HT - 1),
                                )
                            qst = a1_st.tile([128, ST_A], F32, tag="qst")
                            _rope_epilogue(
                                nc, a1_st, ps, qst[:],
                                cos_sb[:, ssl], sin_sb[:, ssl], ST_A,
                            )
                            nc.sync.dma_start(
                                out=qT_dram[h, :, :][:, ssl], in_=qst[:]
                            )

            # ---- Phases B+C pools (attnT + Wo live across both)
            with tc.tile_pool(name="bc", bufs=1) as bc:
                attnT_sb = bc.tile([128, HEADS, S], F32, tag="attnT")
                wo_sb = bc.tile([128, HEADS, H], F32, tag="wo")
                nc.sync.dma_start(out=wo_sb[:], in_=woT_t)

                # ---- Phase B: attention per (head, q-tile), causal tiles only
                with tc.tile_pool(name="b", bufs=1) as b, \
                     tc.tile_pool(name="b_qt", bufs=2) as b_qt, \
                     tc.tile_pool(name="b_p", bufs=3) as b_p, \
                     tc.tile_pool(name="b_r", bufs=2) as b_r, \
                     tc.tile_pool(name="b_sps", bufs=2, space="PSUM") as b_sps, \
                     tc.tile_pool(name="b_ops", bufs=2, space="PSUM") as b_ops, \
                     tc.tile_pool(name="b_dps", bufs=2, space="PSUM") as b_dps, \
                     tc.tile_pool(name="b_bps", bufs=2, space="PSUM") as b_bps:
                    masks_sb = b.tile([128, 4 * QT_W], F32, tag="masks")
                    nc.sync.dma_start(out=masks_sb[:], in_=masks[:])

                    for h in range(HEADS):
                        for qi in range(N_QT):
                            qsl = bass.ts(qi, QT_W)
                            qt = b_qt.tile([128, QT_W], F32, tag="qt")
                            nc.sync.dma_start(
                                out=qt[:], in_=qT_dram[h, :, :][:, qsl]
                            )
                            n_k = 4 * qi + 4
                            out_ps = b_ops.tile([128, QT_W], F32, tag="out")
                            den_ps = b_dps.tile([1, QT_W], F32, tag="den")
                            for ki in range(n_k):
                                s_ps = b_sps.tile([128, QT_W], F32, tag="s")
                                nc.tensor.matmul(
                                    s_ps[:],
                                    kt_sb[:, bass.ts(ki, 128)],
                                    qt[:],
                                    start=True, stop=True,
                                )
                                p_t = b_p.tile([128, QT_W], F32, tag="p")
                                nc.scalar.activation(
                                    p_t[:], s_ps[:],
                                    mybir.ActivationFunctionType.Exp,
                                    scale=SM_SCALE,
                                )
                                off = ki - 4 * qi
                                if off >= 0:
                                    nc.vector.tensor_mul(
                                        p_t[:], p_t[:],
                                        masks_sb[:, bass.ts(off, QT_W)],
                                    )
                                nc.tensor.matmul(
                                    out_ps[:], v_sb[:, ki, :], p_t[:],
                                    start=(ki == 0), stop=(ki == n_k - 1),
                                )
                                nc.tensor.matmul(
                                    den_ps[:], ones_sb[:, 0:1], p_t[:],
                                    start=(ki == 0), stop=(ki == n_k - 1),
                                )
                            recip = b_r.tile([1, QT_W], F32, tag="recip")
                            nc.vector.reciprocal(recip[:], den_ps[:])
                            bc_ps = b_bps.tile([128, QT_W], F32, tag="bc")
                            nc.tensor.matmul(
                                bc_ps[:], ones_sb[0:1, :], recip[:],
                                start=True, stop=True,
                            )
                            bc_sb = b_r.tile([128, QT_W], F32, tag="bcs")
                            nc.scalar.copy(bc_sb[:], bc_ps[:])
                            nc.vector.tensor_mul(
                                attnT_sb[:, h, qsl], out_ps[:], bc_sb[:]
                            )

                # ---- Phase C: partial output projection
                with tc.tile_pool(name="c_ps", bufs=4, space="PSUM") as c_ps, \
                     tc.tile_pool(name="c_st", bufs=4) as c_st:
                    for si in range(S // 128):
                        for ei in range(H // ET):
                            o_ps = c_ps.tile([128, ET], F32, tag="o")
                            for j in range(HEADS):
                                nc.tensor.matmul(
                                    o_ps[:],
                                    attnT_sb[:, j, bass.ts(si, 128)],
                                    wo_sb[:, j, bass.ts(ei, ET)],
                                    start=(j == 0), stop=(j == HEADS - 1),
                                )
                            o_st = c_st.tile([128, ET], F32, tag="ost")
                            nc.vector.tensor_copy(o_st[:], o_ps[:])
                            nc.sync.dma_start(
                                out=out[:][
                                    bass.ts(si, 128), bass.ts(ei, ET)
                                ],
                                in_=o_st[:],
                            )
    _split_sync_waits(nc)
    return nc


_NC_CACHE = None


def _get_program():
    global _NC_CACHE
    if _NC_CACHE is None:
        _NC_CACHE = _build_program()
    return _NC_CACHE


def _host_tables(position_ids):
    pos = position_ids.reshape(-1).astype(np.float32)  # [S]
    inv_freq = (
        1.0
        / (np.float32(ROPE_THETA) ** (np.arange(0, D, 2, dtype=np.float32) / np.float32(D)))
    ).astype(np.float32)  # [64]
    freqs = pos[None, :] * inv_freq[:, None]  # [64, S]
    ang = np.concatenate([freqs, freqs], axis=0)  # [128, S]
    cosT = np.cos(ang).astype(np.float32)
    sinT = np.sin(ang).astype(np.float32)
    sinT[0:64, :] *= -1.0  # sign-fold for rotate_half

    masks = np.zeros((128, 4 * QT_W), dtype=np.float32)
    for off in range(4):
        p = np.arange(128)[:, None]
        c = np.arange(QT_W)[None, :]
        masks[:, off * QT_W : (off + 1) * QT_W] = (128 * off + p <= c).astype(
            np.float32
        )
    return cosT, sinT, masks


def _prepare_in_maps(hidden_states, Wq, Wk, Wv, Wo, position_ids):
    x = np.asarray(hidden_states, dtype=np.float32).reshape(S, H)
    Wq = np.asarray(Wq, dtype=np.float32)
    Wk = np.asarray(Wk, dtype=np.float32)
    Wv = np.asarray(Wv, dtype=np.float32)
    Wo = np.asarray(Wo, dtype=np.float32)

    xT = np.ascontiguousarray(x.T)  # [H, S]
    cosT, sinT, masks = _host_tables(np.asarray(position_ids))
    ident = np.eye(128, dtype=np.float32)
    ones = np.ones((128, 128), dtype=np.float32)

    in_maps = []
    for c in range(N_CORES):
        qs = slice(DQ * c, DQ * (c + 1))
        ks = slice(DKV * c, DKV * (c + 1))
        in_maps.append(
            {
                "xT": xT,
                "wqT": np.ascontiguousarray(Wq[qs, :].T),
                "wkT": np.ascontiguousarray(Wk[ks, :].T),
                "wvT": np.ascontiguousarray(Wv[ks, :].T),
                "woT": np.ascontiguousarray(Wo[:, qs].T),
                "cosT": cosT,
                "sinT": sinT,
                "masks": masks,
                "ident": ident,
                "ones": ones,
            }
        )
    return in_maps


def _finalize(results, batch):
    out = np.zeros((S, H), dtype=np.float32)
    for c in range(N_CORES):
        out += results[c]["out"]
    return out.reshape(batch, S, H)


def kernel(hidden_states, Wq, Wk, Wv, Wo, position_ids):
    from concourse.bass_utils import run_bass_kernel_spmd

    B = hidden_states.shape[0]
    in_maps = _prepare_in_maps(hidden_states, Wq, Wk, Wv, Wo, position_ids)
    nc = _get_program()
    res = run_bass_kernel_spmd(nc, in_maps, list(range(N_CORES)))
    return _finalize(res.results, B)
